# revision 8
# baseline (speedup 1.0000x reference)
"""MeshConv GNN message-passing kernel for 8 TRN2 NeuronCores.

Reference computation (E=500000 edges, C=64 ch, OUT=128):
    n = x[clip(nb)]                          # (E, 4, C) gather
    feat = [x, min(n0,n1), max(n0,n1), min(n2,n3), max(n2,n3)]  # (E, 320)
    h = feat @ W.T                           # (E, 128)
    h = BatchNorm(h, training)  (global batch stats over E)
    out = relu(h)

Strategy: shard E across 8 cores (62500 edges each); x replicated in each
core's DRAM; neighbor rows fetched by indirect DMA (256B/row descriptors);
feat transposed to channel-major via PE transpose; GEMM with W.T chunks
stationary producing h in [out_ch, edge] layout; BN stats accumulated with
ACT accum_out and all-reduced across cores with a 1KB collective; phase B
applies the affine+ReLU per-partition and PE-transposes back to edge-major
for contiguous output writes.  h is held in SBUF (bf16) between phases.
"""

import numpy as np

import concourse.bass as bass
import concourse.bacc as bacc
import concourse.tile as tile
from concourse import mybir
from concourse.bass_utils import run_bass_kernel_spmd
from concourse.masks import make_identity

E, C, OUT = 500000, 64, 128
NCORES = 8
ES = E // NCORES            # 62500 edges per core
P = 128
GROUP = 512                 # edges per matmul group (PSUM bank = [128, 512] f32)
NSUB = GROUP // P           # 4 subtiles per group
NG = (ES + GROUP - 1) // GROUP   # 123 groups (122 full + 36-edge remainder)
ES_PAD = NG * GROUP         # 62976
NIDX = 5                    # self + 4 neighbors per edge
EPS = 1e-5

FP32 = mybir.dt.float32
BF16 = mybir.dt.bfloat16
INT32 = mybir.dt.int32


def _valid_edges(g, es=ES):
    return min(GROUP, es - g * GROUP)



def _copy(nc, use_scalar, out, in_):
    if use_scalar:
        nc.scalar.copy(out=out, in_=in_)
    else:
        nc.vector.tensor_copy(out=out, in_=in_)

def build_kernel(es=ES):
    ng = (es + GROUP - 1) // GROUP
    es_pad = ng * GROUP
    nc = bacc.Bacc("TRN2", num_devices=NCORES)

    x_t = nc.dram_tensor("x", [E, C], FP32, kind="ExternalInput")
    xself_t = nc.dram_tensor("xself", [es_pad, C], FP32, kind="ExternalInput")
    idx_t = nc.dram_tensor("idx", [P, ng * NSUB * NIDX], INT32, kind="ExternalInput")
    wt_t = nc.dram_tensor("wt", [3, P, OUT], FP32, kind="ExternalInput")
    gb_t = nc.dram_tensor("gb", [P, 2], FP32, kind="ExternalInput")
    out_t = nc.dram_tensor("out", [es, OUT], FP32, kind="ExternalOutput")

    cc_in = nc.dram_tensor("cc_in", [P, 2], FP32, kind="Internal")
    cc_out = nc.dram_tensor("cc_out", [P, 2], FP32, kind="Internal", addr_space="Shared")

    with tile.TileContext(nc) as tc:
        with (
            tc.tile_pool(name="singles", bufs=1) as singles,
            tc.tile_pool(name="stage", bufs=3) as stage,
            tc.tile_pool(name="featp", bufs=3) as featp,
            tc.tile_pool(name="featTp", bufs=3) as featTp,
            tc.tile_pool(name="hnp", bufs=3) as hnp,
            tc.tile_pool(name="obp", bufs=4) as obp,
            tc.tile_pool(name="psumT", bufs=4, space="PSUM") as psumT,
            tc.tile_pool(name="psumH", bufs=2, space="PSUM") as psumH,
            tc.tile_pool(name="psumB", bufs=2, space="PSUM") as psumB,
        ):
            # ---- constants / persistent state ----
            ident = singles.tile([P, P], FP32)
            make_identity(nc, ident[:])
            wt_sb = singles.tile([P, 3, OUT], FP32)
            nc.sync.dma_start(out=wt_sb[:], in_=wt_t[:, :, :].rearrange("c p o -> p c o"))
            gb_sb = singles.tile([P, 2], FP32)
            nc.sync.dma_start(out=gb_sb[:], in_=gb_t[:, :])
            idx_sb = singles.tile([P, ng * NSUB * NIDX], INT32)
            nc.sync.dma_start(out=idx_sb[:], in_=idx_t[:, :])

            h_sb = singles.tile([P, es_pad], BF16)
            s1parts = singles.tile([P, P], FP32)
            s2parts = singles.tile([P, P], FP32)
            nc.vector.memset(s1parts[:], 0.0)
            nc.vector.memset(s2parts[:], 0.0)

            # ---- phase A: gather -> feat -> transpose -> GEMM -> stats ----
            for g in range(ng):
                ne = _valid_edges(g, es)

                stg = stage.tile([P, NSUB, NIDX, C], FP32)
                # self rows: contiguous [j*P+p] -> stg[:, j, 0, :]
                nc.sync.dma_start(
                    out=stg[:, :, 0, :],
                    in_=xself_t[g * GROUP:(g + 1) * GROUP, :].rearrange(
                        "(a p) c -> p a c", p=P),
                )
                # neighbor rows: one indirect DMA per (subtile, slot)
                for j in range(NSUB):
                    for r in range(1, NIDX):
                        nc.gpsimd.indirect_dma_start(
                            out=stg[:, j, r, :],
                            out_offset=None,
                            in_=x_t[:, :],
                            in_offset=bass.IndirectOffsetOnAxis(
                                ap=idx_sb[:, (g * NSUB + j) * NIDX + r:
                                          (g * NSUB + j) * NIDX + r + 1],
                                axis=0,
                            ),
                        )

                # feat chunks 0/1 hold the pairwise min/max outputs:
                #   chunk0 = [p1_lo | p1_hi], chunk1 = [p2_lo | p2_hi]
                # chunk2 (self x, 64 ch) is read straight from stg.
                featc = featp.tile([P, 2, NSUB, P], FP32)
                nc.vector.tensor_tensor(
                    out=featc[:, 0, :, 0:C], in0=stg[:, :, 1, :], in1=stg[:, :, 2, :],
                    op=mybir.AluOpType.min)
                nc.vector.tensor_tensor(
                    out=featc[:, 0, :, C:2 * C], in0=stg[:, :, 1, :], in1=stg[:, :, 2, :],
                    op=mybir.AluOpType.max)
                nc.vector.tensor_tensor(
                    out=featc[:, 1, :, 0:C], in0=stg[:, :, 3, :], in1=stg[:, :, 4, :],
                    op=mybir.AluOpType.min)
                nc.vector.tensor_tensor(
                    out=featc[:, 1, :, C:2 * C], in0=stg[:, :, 3, :], in1=stg[:, :, 4, :],
                    op=mybir.AluOpType.max)

                # transpose feat to channel-major: featT[:, c, j, :] = feat_cj.T
                featT = featTp.tile([P, 3, NSUB, P], FP32)
                for j in range(NSUB):
                    for c in range(2):
                        pt = psumT.tile([P, P], FP32, tag="pt")
                        nc.tensor.transpose(
                            out=pt[:], in_=featc[:, c, j, :], identity=ident[:])
                        _copy(nc, (j + c) % 2 == 0, featT[:, c, j, :], pt[:])
                    pt = psumT.tile([P, P], FP32, tag="pt")
                    nc.tensor.transpose(
                        out=pt[0:C, :], in_=stg[:, j, 0, :], identity=ident[:])
                    _copy(nc, j % 2 == 0, featT[0:C, 2, j, :], pt[0:C, :])

                # GEMM: h[o, e] = sum_c WT[c, o] * featT[c, e]
                hp = psumH.tile([P, GROUP], FP32)
                nc.tensor.matmul(
                    out=hp[:], lhsT=wt_sb[:, 0, :], rhs=featT[:, 0, :, :],
                    start=True, stop=False)
                nc.tensor.matmul(
                    out=hp[:], lhsT=wt_sb[:, 1, :], rhs=featT[:, 1, :, :],
                    start=False, stop=False)
                nc.tensor.matmul(
                    out=hp[:], lhsT=wt_sb[0:C, 2, :], rhs=featT[0:C, 2, :, :],
                    start=False, stop=True)

                # store h (bf16) + accumulate per-channel sum / sum-of-squares
                nc.scalar.activation(
                    out=h_sb[:, g * GROUP:g * GROUP + ne], in_=hp[:, 0:ne],
                    func=mybir.ActivationFunctionType.Copy,
                    accum_out=s1parts[:, g:g + 1])
                hsq = stage.tile([P, GROUP], BF16, tag="hsq")
                nc.scalar.activation(
                    out=hsq[:, 0:ne], in_=hp[:, 0:ne],
                    func=mybir.ActivationFunctionType.Square,
                    accum_out=s2parts[:, g:g + 1])

            # ---- stats all-reduce + affine params ----
            S = singles.tile([P, 2], FP32)
            nc.vector.reduce_sum(out=S[:, 0:1], in_=s1parts[:], axis=mybir.AxisListType.X)
            nc.vector.reduce_sum(out=S[:, 1:2], in_=s2parts[:], axis=mybir.AxisListType.X)
            nc.sync.dma_start(out=cc_in[:, :], in_=S[:])
            nc.gpsimd.collective_compute(
                "AllReduce",
                mybir.AluOpType.add,
                ins=[cc_in[:, :]],
                outs=[cc_out[:, :]],
                replica_groups=[list(range(NCORES))],
            )
            Sg = singles.tile([P, 2], FP32)
            nc.sync.dma_start(out=Sg[:], in_=cc_out[:, :])

            prm = singles.tile([P, 6], FP32)
            mean, ex2, var, rstd, scl, bias = (prm[:, i:i + 1] for i in range(6))
            nc.scalar.mul(mean, Sg[:, 0:1], 1.0 / (es * NCORES))
            nc.scalar.mul(ex2, Sg[:, 1:2], 1.0 / (es * NCORES))
            nc.vector.tensor_tensor(out=var, in0=mean, in1=mean, op=mybir.AluOpType.mult)
            nc.vector.tensor_tensor(out=var, in0=ex2, in1=var, op=mybir.AluOpType.subtract)
            sd = singles.tile([P, 1], FP32)
            eps_sb = singles.tile([P, 1], FP32)
            nc.vector.memset(eps_sb[:], EPS)
            nc.scalar.activation(out=sd[:], in_=var, func=mybir.ActivationFunctionType.Sqrt,
                                 bias=eps_sb[:])
            nc.vector.reciprocal(out=rstd, in_=sd[:])
            nc.vector.tensor_tensor(out=scl, in0=gb_sb[:, 0:1], in1=rstd, op=mybir.AluOpType.mult)
            nc.vector.tensor_tensor(out=bias, in0=mean, in1=scl, op=mybir.AluOpType.mult)
            nc.vector.tensor_tensor(out=bias, in0=gb_sb[:, 1:2], in1=bias, op=mybir.AluOpType.subtract)

            # ---- phase B: affine+ReLU, transpose to edge-major, write out ----
            for g in range(ng):
                ne = _valid_edges(g, es)
                hn = hnp.tile([P, GROUP], FP32)
                nc.scalar.activation(
                    out=hn[:, 0:ne], in_=h_sb[:, g * GROUP:g * GROUP + ne],
                    func=mybir.ActivationFunctionType.Relu,
                    bias=bias, scale=scl)
                for j in range((ne + P - 1) // P):
                    nr = min(P, ne - j * P)
                    pb = psumB.tile([P, P], FP32, tag="pb")
                    nc.tensor.transpose(
                        out=pb[0:nr, :], in_=hn[:, j * P:j * P + nr], identity=ident[:])
                    ob = obp.tile([P, P], FP32)
                    _copy(nc, j % 2 == 0, ob[0:nr, :], pb[0:nr, :])
                    e0 = g * GROUP + j * P
                    nc.sync.dma_start(out=out_t[e0:e0 + nr, :], in_=ob[0:nr, :])

    nc.compile()
    return nc


def prep_inputs(x, nb, W, gamma, beta, es=ES):
    """Host-side layout prep: per-core index arrays, W.T chunks, gamma/beta pack."""
    idx = np.clip(nb, 0, E - 1).astype(np.int64)

    # WT rows reordered to feat order [p1_lo, p1_hi, p2_lo, p2_hi, x]:
    # chunks: 0 -> [W_p1lo; W_p1hi], 1 -> [W_p2lo; W_p2hi], 2 -> [W_x; pad]
    WT = np.ascontiguousarray(W.T.astype(np.float32))     # [320, 128]
    wt = np.zeros((3, P, OUT), dtype=np.float32)
    wt[0] = WT[C:3 * C]
    wt[1] = WT[3 * C:5 * C]
    wt[2, 0:C] = WT[0:C]

    gb = np.stack([gamma.astype(np.float32), beta.astype(np.float32)], axis=1)

    ng = (es + GROUP - 1) // GROUP
    es_pad = ng * GROUP
    in_maps = []
    for c in range(NCORES):
        base = c * ES
        sl = idx[base:base + es]                           # [es, 4]
        padded = np.zeros((es_pad, NIDX), dtype=np.int64)
        padded[:es, 0] = np.arange(base, base + es)        # self index
        padded[es:, 0] = base
        padded[:es, 1:] = sl
        padded[es:, 1:] = base
        # [g*GROUP + j*P + p, r] -> A[p, (g, j, r)]
        A = padded.reshape(ng, NSUB, P, NIDX).transpose(2, 0, 1, 3)
        A = np.ascontiguousarray(A.reshape(P, ng * NSUB * NIDX), dtype=np.int32)
        xs = np.zeros((es_pad, C), dtype=np.float32)
        xs[:es] = x[base:base + es]
        in_maps.append({
            "x": np.ascontiguousarray(x, dtype=np.float32),
            "xself": xs,
            "idx": A,
            "wt": wt,
            "gb": gb,
        })
    return in_maps


_NC_CACHE = {}


def kernel(x, nb, W, gamma, beta, _trace=False):
    x = np.asarray(x)
    nb = np.asarray(nb)
    W = np.asarray(W)
    gamma = np.asarray(gamma)
    beta = np.asarray(beta)

    if "nc" not in _NC_CACHE:
        _NC_CACHE["nc"] = build_kernel()
    nc = _NC_CACHE["nc"]

    in_maps = prep_inputs(x, nb, W, gamma, beta)
    res = run_bass_kernel_spmd(
        nc, in_maps, core_ids=list(range(NCORES)), trace=_trace,
    )
    out = np.concatenate([r["out"] for r in res.results], axis=0)
    _NC_CACHE["last_result"] = res
    return out
